# revision 11
# baseline (speedup 1.0000x reference)
"""Channel-attention (XCA-style) Trainium2 kernel, 8-core SPMD, v4.

Sharding: spatial row-bands (32 rows/core + 1-row halo), both batches on
every core. Cross-core coupling is only the per-(batch,head) q/k Gram
block and squared norms, all-reduced as fp32 per batch.

v4 changes vs v3 (782us measured):
- depthwise split: the 3 dx=0 taps (2-byte-misaligned for DVE) run on
  the PE as diagonal matmuls accumulating in PSUM, evacuated by Scalar
  as the qt init; DVE adds only the 6 aligned taps as ts_mul+tt_add
  pairs (scalar_tensor_tensor measured 1x-mode, reverted).
- gram/post use no DVE at all (Scalar rsqrt/exp/copies, GpSimd tensor
  ops with fused accum_out row-sums, PE matmuls), so the in-order DVE
  queue runs the two dw blocks back-to-back and never gates post/final.
- softmax max-subtraction dropped (logits = temperature * cosine sims,
  |logit| <= temp; exp is safe).
- phase order: qk0 dwPE0 conv0 qk1 dwPE1 gram0 conv1 post0 gram1 post1
  final0 final1, with v double-buffered so conv1 overlaps the gram/AR
  critical path.
"""
import os
import sys

sys.path.insert(0, '/opt/trn_rl_repo')

import numpy as np
import ml_dtypes

import concourse.bass as bass
import concourse.bacc as bacc
import concourse.tile as tile
import concourse.mybir as mybir
from concourse.bass_utils import run_bass_kernel_spmd

BF16 = mybir.dt.bfloat16
F32 = mybir.dt.float32
ADD = mybir.AluOpType.add
MULT = mybir.AluOpType.mult

N_CORES = 8
DIM = 192
HEADS = 4
HD = DIM // HEADS            # 48
UD = 2 * HD                  # 96 channels per head in u=[q_h;k_h] order
B = 2
H = 256
W = 256
ROWS = H // N_CORES          # 32 output rows per core
RIN = ROWS + 2               # input band rows (halo)
WG = W + 2                   # guarded width
NI = ROWS * W                # 8192 interior positions per batch
NF = RIN * W                 # 8704 band positions per batch
NB = 512                     # matmul N-chunk
NCH = NI // NB               # 16 conv/final chunks
TAPS = [(dy, dx) for dy in (-1, 0, 1) for dx in (-1, 0, 1)]
PE_TAPS = [1, 4, 7]          # dx == 0 taps, done on PE
DVE_TAPS = [t for t in range(9) if t not in PE_TAPS]

# segments of qk tiles covering u-channels [96h, 96h+96) : (tile, lo, hi)
# XBAR transpose DMA sources starting at partition > 0 are limited to 32
# partitions, so non-zero-base segments are split into 32-row chunks.
def _segs(raw):
    out = []
    for (ct, lo, hi) in raw:
        if lo == 0:
            out.append((ct, lo, hi))
        else:
            out += [(ct, s, min(s + 32, hi)) for s in range(lo, hi, 32)]
    return out


HEAD_SEGS = {
    0: _segs([(0, 0, 96)]),
    1: _segs([(0, 96, 128), (1, 0, 64)]),
    2: _segs([(1, 64, 128), (2, 0, 32)]),
    3: _segs([(2, 32, 128)]),
}

LAST_RESULTS = None
_CACHED_NC = None


def _u_perm():
    p = []
    for h in range(HEADS):
        p += list(range(h * HD, (h + 1) * HD))
        p += list(range(DIM + h * HD, DIM + (h + 1) * HD))
    return np.array(p)


def _bf16(a):
    return np.ascontiguousarray(a.astype(ml_dtypes.bfloat16))


def build_nc():
    nc = bacc.Bacc("TRN2", target_bir_lowering=False, debug=False,
                   enable_asserts=False, num_devices=N_CORES)
    xb = nc.dram_tensor("xb", [B, DIM, NF], BF16, kind="ExternalInput").ap()
    yb = nc.dram_tensor("yb", [B, DIM, NF], BF16, kind="ExternalInput").ap()
    wqk = nc.dram_tensor("wqk", [DIM, 2 * DIM], BF16, kind="ExternalInput").ap()
    aqk = nc.dram_tensor("aqk", [3, 128, 9], F32, kind="ExternalInput").ap()
    ddw = nc.dram_tensor("ddw", [128, 9, 128], BF16, kind="ExternalInput").ap()
    wconv = nc.dram_tensor("wconv", [9, DIM, DIM], BF16, kind="ExternalInput").ap()
    projth = nc.dram_tensor("projth", [HEADS, HD, DIM], BF16, kind="ExternalInput").ap()
    tempt = nc.dram_tensor("tempt", [HD, HEADS], F32, kind="ExternalInput").ap()
    eye = nc.dram_tensor("eye", [UD, UD], F32, kind="ExternalInput").ap()
    out = nc.dram_tensor("out", [B, DIM, NI], BF16, kind="ExternalOutput").ap()

    with tile.TileContext(nc) as tc:
        with tc.tile_pool(name="wpool", bufs=1) as wp, \
             tc.tile_pool(name="xs", bufs=2) as xsp, \
             tc.tile_pool(name="mid", bufs=1) as midp, \
             tc.tile_pool(name="yt", bufs=1) as ytp, \
             tc.tile_pool(name="qk", bufs=1) as qkp, \
             tc.tile_pool(name="dwt", bufs=1) as dwp, \
             tc.tile_pool(name="ut", bufs=2) as utp, \
             tc.tile_pool(name="vv", bufs=1) as vvp, \
             tc.tile_pool(name="small", bufs=1) as smp, \
             tc.tile_pool(name="ost", bufs=3) as ostp, \
             tc.tile_pool(name="psA", bufs=4, space="PSUM") as psA, \
             tc.tile_pool(name="psB", bufs=2, space="PSUM") as psB, \
             tc.tile_pool(name="psG", bufs=1, space="PSUM") as psG, \
             tc.tile_pool(name="psS", bufs=1, space="PSUM") as psS, \
             tc.tile_pool(name="dram", bufs=2, space="DRAM") as drp:

            # ---- persistent weights ----
            wqk_a = wp.tile([128, 2 * DIM], BF16, tag="wqk_a")
            wqk_b = wp.tile([64, 2 * DIM], BF16, tag="wqk_b")
            nc.sync.dma_start(wqk_a[:], wqk[0:128, :])
            nc.sync.dma_start(wqk_b[:], wqk[128:192, :])
            wca = wp.tile([128, 9, DIM], BF16, tag="wca")
            wcb = wp.tile([64, 9, DIM], BF16, tag="wcb")
            nc.sync.dma_start(wca[:], wconv[:, 0:128, :].rearrange("t k m -> k t m"))
            nc.sync.dma_start(wcb[:],
                              wconv[:, 128:192, :].rearrange("t k m -> k t m"))
            aqk_sb = wp.tile([128, 3, 9], F32, tag="aqk")
            nc.sync.dma_start(aqk_sb[:], aqk.rearrange("c k t -> k c t"))
            ddw_sb = wp.tile([128, 9, 128], BF16, tag="ddw")
            nc.sync.dma_start(ddw_sb[:], ddw[:])
            projth_sb = wp.tile([HD, HEADS, DIM], BF16, tag="projth")
            nc.sync.dma_start(projth_sb[:], projth.rearrange("h d f -> d h f"))
            tempt_sb = wp.tile([HD, HEADS], F32, tag="tempt")
            nc.sync.dma_start(tempt_sb[:], tempt[:])
            eye_sb = wp.tile([UD, UD], F32, tag="eye")
            nc.sync.dma_start(eye_sb[:], eye[:])
            ones_sb = wp.tile([UD, UD], F32, tag="ones")
            nc.vector.memset(ones_sb[:], 1.0)

            GRP = 512
            groups = [(g, min(g + GRP, NF)) for g in range(0, NF, GRP)]

            # guarded tiles are allocated once and reused for both batches;
            # guard columns stay zero (interior writes never touch them).
            mids = []
            for ct in range(3):
                mid = midp.tile([128, RIN, WG], BF16, tag=f"mid{ct}")
                nc.gpsimd.memset(mid[:, :, 0:1], 0.0)
                nc.gpsimd.memset(mid[:, :, WG - 1:WG], 0.0)
                mids.append(mid)
            ya = ytp.tile([128, RIN, WG], BF16, tag="ya")
            ybt = ytp.tile([64, RIN, WG], BF16, tag="ybt")
            nc.gpsimd.memset(ya[:, :, 0:1], 0.0)
            nc.gpsimd.memset(ya[:, :, WG - 1:WG], 0.0)
            nc.gpsimd.memset(ybt[:, :, 0:1], 0.0)
            nc.gpsimd.memset(ybt[:, :, WG - 1:WG], 0.0)

            def qk_mm_phase(b):
                """1x1 conv into 3 guarded mid tiles (PE + scalar evac)."""
                for (g0, g1) in groups:
                    x_a = xsp.tile([128, GRP], BF16, tag="x_a")
                    x_b = xsp.tile([64, GRP], BF16, tag="x_b")
                    nc.sync.dma_start(x_a[:, 0:g1 - g0], xb[b, 0:128, g0:g1])
                    nc.sync.dma_start(x_b[:, 0:g1 - g0], xb[b, 128:192, g0:g1])
                    for nb in range(g0 // NB, g1 // NB):
                        ns = slice(nb * NB - g0, (nb + 1) * NB - g0)
                        pss = [psA.tile([128, NB], F32, tag="psA",
                                        name=f"psqk{m}") for m in range(3)]
                        for m in range(3):
                            mcol = slice(m * 128, (m + 1) * 128)
                            nc.tensor.matmul(pss[m][:], wqk_a[:, mcol],
                                             x_a[:, ns], start=True, stop=False)
                        for m in range(3):
                            mcol = slice(m * 128, (m + 1) * 128)
                            nc.tensor.matmul(pss[m][:], wqk_b[:, mcol],
                                             x_b[:, ns], start=False, stop=True)
                        r = nb * 2
                        for m in range(3):
                            nc.scalar.copy(
                                mids[m][:, r:r + 2, 1:W + 1],
                                pss[m][:].rearrange("p (r w) -> p r w", r=2))

            def dw_pe_phase(b):
                """dx=0 depthwise taps as diagonal matmuls; scalar evac is
                the qt init."""
                qkt = []
                for ct in range(3):
                    qt = qkp.tile([128, ROWS, W], BF16, tag=f"qk{ct}")
                    for j in range(NCH):
                        psD = psB.tile([128, NB], F32, tag="psB", name="psD")
                        for k, t in enumerate(PE_TAPS):
                            dy = TAPS[t][0]
                            rs = slice(1 + 2 * j + dy, 3 + 2 * j + dy)
                            nc.tensor.matmul(psD[:], ddw_sb[:, ct * 3 + k, :],
                                             mids[ct][:, rs, 1:W + 1],
                                             start=(k == 0), stop=(k == 2))
                        nc.scalar.copy(
                            qt[:, 2 * j:2 * j + 2, :],
                            psD[:].rearrange("p (r w) -> p r w", r=2))
                    qkt.append(qt)
                return qkt

            def dw_dve_phase(b, qkt):
                """6 aligned depthwise taps on DVE: ts_mul + tt_add."""
                for ct in range(3):
                    mid, qt = mids[ct], qkt[ct]
                    dwt = dwp.tile([128, ROWS // 4, W], BF16, tag="dwt")
                    for hf in range(4):
                        ro = slice(hf * 8, hf * 8 + 8)
                        for t in DVE_TAPS:
                            dy, dx = TAPS[t]
                            rs = slice(1 + hf * 8 + dy, 9 + hf * 8 + dy)
                            src = mid[:, rs, 1 + dx:1 + W + dx]
                            nc.vector.tensor_scalar_mul(
                                dwt[:], src, aqk_sb[:, ct, t:t + 1])
                            nc.vector.tensor_tensor(
                                qt[:, ro, :], dwt[:], qt[:, ro, :], op=ADD)

            def conv_phase(b):
                """3x3 full conv y -> v (192ch) with folded 1x1 weights.
                Pass A: out 0:128 per chunk. Pass B: out 128:192, chunk
                pairs col-packed into one PSUM tile."""
                nc.sync.dma_start(
                    ya[:, :, 1:W + 1],
                    yb[b, 0:128, :].rearrange("c (r w) -> c r w", r=RIN))
                nc.sync.dma_start(
                    ybt[:, :, 1:W + 1],
                    yb[b, 128:192, :].rearrange("c (r w) -> c r w", r=RIN))
                v0 = vvp.tile([128, NI], BF16, tag="v0")
                vB = vvp.tile([128, NI // 2], BF16, tag="vB")

                def win(j, t):
                    dy, dx = TAPS[t]
                    return (slice(1 + 2 * j + dy, 3 + 2 * j + dy),
                            slice(1 + dx, 1 + W + dx))

                for j in range(NCH):
                    ns = slice(j * NB, (j + 1) * NB)
                    psP = psA.tile([128, NB], F32, tag="psA", name="psP")
                    for t in range(9):
                        rs, cs = win(j, t)
                        nc.tensor.matmul(psP[:], wca[:, t, 0:128],
                                         ya[:, rs, cs],
                                         start=(t == 0), stop=False)
                    for t in range(9):
                        rs, cs = win(j, t)
                        nc.tensor.matmul(psP[:], wcb[:, t, 0:128],
                                         ybt[:, rs, cs],
                                         start=False, stop=(t == 8))
                    nc.scalar.copy(v0[:, ns], psP[:])
                for p in range(NCH // 2):
                    j0, j1 = 2 * p, 2 * p + 1
                    pns = slice(p * NB, (p + 1) * NB)
                    psR = psB.tile([128, NB], F32, tag="psB", name="psR")
                    for t in range(9):
                        rs0, cs0 = win(j0, t)
                        rs1, cs1 = win(j1, t)
                        nc.tensor.matmul(psR[0:64, :], wca[:, t, 128:192],
                                         ya[:, rs0, cs0],
                                         start=(t == 0), stop=False)
                        nc.tensor.matmul(psR[64:128, :], wca[:, t, 128:192],
                                         ya[:, rs1, cs1],
                                         start=(t == 0), stop=False)
                    for t in range(9):
                        rs0, cs0 = win(j0, t)
                        rs1, cs1 = win(j1, t)
                        nc.tensor.matmul(psR[0:64, :], wcb[:, t, 128:192],
                                         ybt[:, rs0, cs0],
                                         start=False, stop=(t == 8))
                        nc.tensor.matmul(psR[64:128, :], wcb[:, t, 128:192],
                                         ybt[:, rs1, cs1],
                                         start=False, stop=(t == 8))
                    nc.scalar.copy(vB[:, pns], psR[:])
                return v0, vB

            def gram_ar_phase(b, qkt):
                """Per-head u-gram + norms; DVE-free (scalar/gpsimd/PE)."""
                gu_all = smp.tile([UD, HEADS, UD], F32, tag="gu_all")
                nrow = smp.tile([1, HEADS, UD], F32, tag="nrow")
                n_bands = NI // 2048
                tp_i = 0
                for h in range(HEADS):
                    gps = psG.tile([128, UD], F32, tag="psG")
                    for band in range(n_bands):
                        rsl = slice(band * 8, (band + 1) * 8)
                        ut = utp.tile([128, 16, 128], BF16, tag="ut")
                        off = 0
                        for (ct, lo, hi) in HEAD_SEGS[h]:
                            eng = nc.sync if tp_i % 2 == 0 else nc.scalar
                            eng.dma_start_transpose(
                                ut[:, :, off:off + hi - lo],
                                qkt[ct][lo:hi, rsl, :])
                            tp_i += 1
                            off += hi - lo
                        for c in range(16):
                            nc.tensor.matmul(
                                gps[:], ut[:, c, :], ut[:, c, 0:UD],
                                start=(band == 0 and c == 0),
                                stop=(band == n_bands - 1 and c == 15))
                    nc.scalar.copy(gu_all[:, h, :], gps[0:UD, :])
                # qk block + diag norms. tmp = gu .* I is diagonal, so its
                # column sums (ones.T @ tmp) are exactly the norms row.
                aru = smp.tile([HD, HEADS, HD], F32, tag="aru")
                tmp = smp.tile([UD, UD], F32, tag="tmp")
                for h in range(HEADS):
                    nc.scalar.copy(aru[:, h, :], gu_all[0:HD, h, HD:UD])
                    nc.gpsimd.tensor_tensor(
                        tmp[:], gu_all[:, h, :], eye_sb[:], op=MULT)
                    kps = psS.tile([1, UD], F32, tag="psS", name="kps")
                    nc.tensor.matmul(kps[:], ones_sb[:, 0:1], tmp[:],
                                     start=True, stop=True)
                    nc.scalar.copy(nrow[:, h, :], kps[:])
                AW = HEADS * UD
                ar_in = drp.tile([HD + 1, AW], F32, tag="ar_in")
                ar_out = drp.tile([HD + 1, AW], F32, tag="ar_out")
                nc.sync.dma_start(ar_in[0:HD, 0:HEADS * HD],
                                  aru[:].rearrange("p h d -> p (h d)"))
                nc.sync.dma_start(ar_in[HD:HD + 1, :],
                                  nrow[:].rearrange("p h d -> p (h d)"))
                nc.gpsimd.collective_compute(
                    "AllReduce", ADD,
                    replica_groups=[list(range(N_CORES))],
                    ins=[ar_in.opt()], outs=[ar_out.opt()])
                gqk = smp.tile([HD, HEADS, HD], F32, tag="gqk")
                nsqr = smp.tile([1, HEADS, UD], F32, tag="nsqr")
                nsqt = smp.tile([UD, HEADS], F32, tag="nsqt")
                nc.sync.dma_start(gqk[:].rearrange("p h d -> p (h d)"),
                                  ar_out[0:HD, 0:HEADS * HD])
                nc.sync.dma_start(nsqr[:].rearrange("p h d -> p (h d)"),
                                  ar_out[HD:HD + 1, :])
                # transposed copy of the norms row -> per-partition layout
                nc.sync.dma_start(nsqt[:],
                                  ar_out[HD:HD + 1, :]
                                  .rearrange("p (h d) -> (p d) h", h=HEADS))
                return gqk, nsqr, nsqt

            def post_phase(b, gqk, nsqr, nsqt):
                """Softmax + proj fold; DVE use limited to 3 tiny
                reciprocals (queued after the dw blocks)."""
                # invu[c, h] = 1/||u_c|| = sqrt(1/nsq): reciprocal on DVE
                # (scalar Rsqrt is banned for accuracy), sqrt on Scalar.
                inq = smp.tile([UD, HEADS], F32, tag="inq")
                nc.vector.reciprocal(inq[:], nsqt[:])
                inqr = smp.tile([1, HEADS, UD], F32, tag="inqr")
                nc.vector.reciprocal(inqr[:], nsqr[:])
                invu = smp.tile([UD, HEADS], F32, tag="invu")
                nc.scalar.sqrt(invu[:], inq[:])
                invrow = smp.tile([1, HEADS, UD], F32, tag="invrow")
                nc.scalar.sqrt(invrow[:], inqr[:])
                invq = smp.tile([HD, HEADS], F32, tag="invq")
                nc.gpsimd.tensor_tensor(invq[:], invu[0:HD, :], tempt_sb[:],
                                        op=MULT)
                exs = smp.tile([HD, HEADS, HD], BF16, tag="exs")
                sm_all = smp.tile([HD, HEADS], F32, tag="sm_all")
                for h in range(HEADS):
                    bc = smp.tile([HD, HD], F32, tag="bc")
                    bps = psS.tile([HD, HD], F32, tag="psS", name="bps")
                    nc.tensor.matmul(bps[:], ones_sb[0:1, 0:HD],
                                     invrow[:, h, HD:UD], start=True, stop=True)
                    nc.scalar.copy(bc[:], bps[:])
                    lg = smp.tile([HD, HD], F32, tag="lg")
                    nc.gpsimd.tensor_scalar_mul(
                        lg[:], gqk[:, h, :], invq[:, h:h + 1])
                    nc.gpsimd.tensor_tensor(lg[:], lg[:], bc[:], op=MULT)
                    # logits bounded by temperature (cosine sims): no max
                    # subtraction needed; fused row-sum via accum_out.
                    nc.scalar.activation(exs[:, h, :], lg[:],
                                         mybir.ActivationFunctionType.Exp,
                                         accum_out=sm_all[:, h:h + 1])
                rs_all = smp.tile([HD, HEADS], F32, tag="rs_all")
                nc.vector.reciprocal(rs_all[:], sm_all[:])
                mst = smp.tile([HD, HEADS, DIM], BF16, tag="mst")
                for h in range(HEADS):
                    mps = psS.tile([HD, DIM], F32, tag="psS", name="mps")
                    nc.tensor.matmul(mps[:], exs[:, h, :], projth_sb[:, h, :],
                                     start=True, stop=True)
                    # softmax 1/rowsum folded into the PSUM evacuation
                    nc.scalar.mul(mst[:, h, :], mps[:], rs_all[:, h:h + 1])
                pe0 = smp.tile([128, DIM], BF16, tag="pe0")
                pe1 = smp.tile([128, DIM], BF16, tag="pe1")
                nc.sync.dma_start(pe0[0:48, :], mst[:, 0, :])
                nc.sync.dma_start(pe0[48:96, :], mst[:, 1, :])
                nc.sync.dma_start(pe0[96:128, :], mst[0:32, 2, :])
                # pe1 duplicated on both partition halves so the K=64
                # final matmuls can run at base partition 0 or 64 to
                # match the col-packed vB layout.
                for base in (0, 64):
                    nc.sync.dma_start(pe1[base:base + 16, :], mst[32:48, 2, :])
                    nc.sync.dma_start(pe1[base + 16:base + 64, :], mst[:, 3, :])
                return pe0, pe1

            def final_phase(b, pe0, pe1, v0, vB):
                """out = PeffT.T @ v.  psF0: out 0:128 per chunk.  psF2:
                out 128:192 col-packed chunk pairs."""
                for p in range(NCH // 2):
                    j0, j1 = 2 * p, 2 * p + 1
                    pns = slice(p * NB, (p + 1) * NB)
                    ns0 = slice(j0 * NB, (j0 + 1) * NB)
                    ns1 = slice(j1 * NB, (j1 + 1) * NB)
                    psF0a = psA.tile([128, NB], F32, tag="psA", name="psF0a")
                    psF0b = psA.tile([128, NB], F32, tag="psA", name="psF0b")
                    psF2 = psB.tile([128, NB], F32, tag="psB", name="psF2")
                    nc.tensor.matmul(psF0a[:], pe0[:, 0:128], v0[:, ns0],
                                     start=True, stop=False)
                    nc.tensor.matmul(psF0a[:], pe1[0:64, 0:128], vB[0:64, pns],
                                     start=False, stop=True)
                    nc.tensor.matmul(psF0b[:], pe0[:, 0:128], v0[:, ns1],
                                     start=True, stop=False)
                    nc.tensor.matmul(psF0b[:], pe1[64:128, 0:128],
                                     vB[64:128, pns], start=False, stop=True)
                    nc.tensor.matmul(psF2[0:64, :], pe0[:, 128:192], v0[:, ns0],
                                     start=True, stop=False)
                    nc.tensor.matmul(psF2[0:64, :], pe1[0:64, 128:192],
                                     vB[0:64, pns], start=False, stop=True)
                    nc.tensor.matmul(psF2[64:128, :], pe0[:, 128:192],
                                     v0[:, ns1], start=True, stop=False)
                    nc.tensor.matmul(psF2[64:128, :], pe1[64:128, 128:192],
                                     vB[64:128, pns], start=False, stop=True)
                    ost0 = ostp.tile([128, NB], BF16, tag="ost")
                    ost1 = ostp.tile([128, NB], BF16, tag="ost")
                    ost2 = ostp.tile([128, NB], BF16, tag="ost")
                    nc.scalar.copy(ost0[:], psF0a[:])
                    nc.scalar.copy(ost1[:], psF0b[:])
                    nc.scalar.copy(ost2[:], psF2[:])
                    nc.sync.dma_start(out[b, 0:128, ns0], ost0[:])
                    nc.sync.dma_start(out[b, 0:128, ns1], ost1[:])
                    nc.sync.dma_start(out[b, 128:192, ns0], ost2[0:64, :])
                    nc.sync.dma_start(out[b, 128:192, ns1], ost2[64:128, :])

            # ---- global schedule ----
            # Per-engine queues are in-order; the DVE queue holds the two
            # dw blocks back-to-back (plus post's 3 tiny reciprocals).
            # final0 precedes conv1 so v (bufs=1) can be reused; the tile
            # framework pipelines conv1 chunk evacs behind final0's
            # per-chunk reads.
            qk_mm_phase(0)
            qkt0 = dw_pe_phase(0)
            dw_dve_phase(0, qkt0)
            v00, vB0 = conv_phase(0)
            qk_mm_phase(1)
            qkt1 = dw_pe_phase(1)
            dw_dve_phase(1, qkt1)
            gqk0, nr0, nt0 = gram_ar_phase(0, qkt0)
            pe00, pe10 = post_phase(0, gqk0, nr0, nt0)
            final_phase(0, pe00, pe10, v00, vB0)
            v01, vB1 = conv_phase(1)
            gqk1, nr1, nt1 = gram_ar_phase(1, qkt1)
            pe01, pe11 = post_phase(1, gqk1, nr1, nt1)
            final_phase(1, pe01, pe11, v01, vB1)

    nc.finalize()
    return nc


def _host_prep(inputs):
    x = np.asarray(inputs["x"], dtype=np.float32)
    y = np.asarray(inputs["y"], dtype=np.float32)
    qk_w = np.asarray(inputs["qk_w"], dtype=np.float32)[:, :, 0, 0]
    qk_dw = np.asarray(inputs["qk_dw_w"], dtype=np.float32)[:, 0]
    v_w = np.asarray(inputs["v_w"], dtype=np.float32)[:, :, 0, 0]
    v_dw = np.asarray(inputs["v_dw_w"], dtype=np.float32)
    proj = np.asarray(inputs["proj_w"], dtype=np.float32)[:, :, 0, 0]
    temp = np.asarray(inputs["temperature"], dtype=np.float32).reshape(HEADS)

    perm = _u_perm()
    wqk_l = _bf16(qk_w[perm].T)                              # [192, 384]
    aqk_t = np.ascontiguousarray(
        qk_dw[perm].reshape(3, 128, 9).astype(np.float32))
    # diagonal weight matrices for the PE depthwise taps (dx == 0)
    ddw_t = np.zeros((128, 9, 128), np.float32)
    for ct in range(3):
        for k, t in enumerate(PE_TAPS):
            np.fill_diagonal(ddw_t[:, ct * 3 + k, :], aqk_t[ct, :, t])
    # fold v 1x1 into the 3x3: v = Conv3x3(y; W_dw@W_v), lhsT layout [in,out]
    wconv_l = _bf16(np.stack(
        [(v_dw[:, :, dy + 1, dx + 1] @ v_w).T for dy, dx in TAPS]))
    projth = _bf16(np.stack(
        [proj[:, h * HD:(h + 1) * HD].T for h in range(HEADS)]))  # [4,48,192]
    tempt = np.ascontiguousarray(
        np.broadcast_to(temp[None, :], (HD, HEADS)).astype(np.float32))
    eye = np.eye(UD, dtype=np.float32)

    # halo-padded row bands per core, bf16
    xp = np.pad(x, ((0, 0), (0, 0), (1, 1), (0, 0)))
    yp = np.pad(y, ((0, 0), (0, 0), (1, 1), (0, 0)))
    shared = dict(wqk=wqk_l, aqk=aqk_t, ddw=_bf16(ddw_t), wconv=wconv_l,
                  projth=projth, tempt=tempt, eye=eye)
    in_maps = []
    for c in range(N_CORES):
        rs = slice(c * ROWS, c * ROWS + RIN)
        in_maps.append(dict(
            xb=_bf16(xp[:, :, rs]).reshape(B, DIM, NF),
            yb=_bf16(yp[:, :, rs]).reshape(B, DIM, NF),
            **shared))
    return in_maps


def kernel(**inputs):
    global LAST_RESULTS, _CACHED_NC
    in_maps = _host_prep(inputs)
    if _CACHED_NC is None:
        _CACHED_NC = build_nc()
    res = run_bass_kernel_spmd(
        _CACHED_NC, in_maps, core_ids=list(range(N_CORES)))
    LAST_RESULTS = res
    out = np.empty((B, DIM, H, W), np.float32)
    for c in range(N_CORES):
        band = res.results[c]["out"].astype(np.float32).reshape(B, DIM, ROWS, W)
        out[:, :, c * ROWS:(c + 1) * ROWS] = band
    return out


# revision 37
# speedup vs baseline: 1.0340x; 1.0340x over previous
"""Channel-attention (XCA-style) Trainium2 kernel, 8-core SPMD, v4.

Sharding: spatial row-bands (32 rows/core + 1-row halo), both batches on
every core. Cross-core coupling is only the per-(batch,head) q/k Gram
block and squared norms, all-reduced as fp32 per batch.

v4 changes vs v3 (782us measured):
- depthwise split: the 3 dx=0 taps (2-byte-misaligned for DVE) run on
  the PE as diagonal matmuls accumulating in PSUM, evacuated by Scalar
  as the qt init; DVE adds only the 6 aligned taps as ts_mul+tt_add
  pairs (scalar_tensor_tensor measured 1x-mode, reverted).
- gram/post use no DVE at all (Scalar rsqrt/exp/copies, GpSimd tensor
  ops with fused accum_out row-sums, PE matmuls), so the in-order DVE
  queue runs the two dw blocks back-to-back and never gates post/final.
- softmax max-subtraction dropped (logits = temperature * cosine sims,
  |logit| <= temp; exp is safe).
- phase order: qk0 dwPE0 conv0 qk1 dwPE1 gram0 conv1 post0 gram1 post1
  final0 final1, with v double-buffered so conv1 overlaps the gram/AR
  critical path.
"""
import os
import sys

sys.path.insert(0, '/opt/trn_rl_repo')

import numpy as np
import ml_dtypes

import concourse.bass as bass
import concourse.bacc as bacc
import concourse.tile as tile
import concourse.mybir as mybir
from concourse.bass_utils import run_bass_kernel_spmd

BF16 = mybir.dt.bfloat16
F32 = mybir.dt.float32
ADD = mybir.AluOpType.add
MULT = mybir.AluOpType.mult

N_CORES = 8
DIM = 192
HEADS = 4
HD = DIM // HEADS            # 48
UD = 2 * HD                  # 96 channels per head in u=[q_h;k_h] order
B = 2
H = 256
W = 256
ROWS = H // N_CORES          # 32 output rows per core
RIN = ROWS + 2               # input band rows (halo)
WG = W + 2                   # guarded width
NI = ROWS * W                # 8192 interior positions per batch
NF = RIN * W                 # 8704 band positions per batch
NB = 512                     # matmul N-chunk
NCH = NI // NB               # 16 conv/final chunks
TAPS = [(dy, dx) for dy in (-1, 0, 1) for dx in (-1, 0, 1)]
PE_TAPS = [1, 4, 7]          # dx == 0 taps, done on PE
DVE_TAPS = [t for t in range(9) if t not in PE_TAPS]

# segments of qk tiles covering u-channels [96h, 96h+96) : (tile, lo, hi)
# XBAR transpose DMA sources starting at partition > 0 are limited to 32
# partitions, so non-zero-base segments are split into 32-row chunks.
def _segs(raw):
    out = []
    for (ct, lo, hi) in raw:
        if lo == 0:
            out.append((ct, lo, hi))
        else:
            out += [(ct, s, min(s + 32, hi)) for s in range(lo, hi, 32)]
    return out


HEAD_SEGS = {
    0: _segs([(0, 0, 96)]),
    1: _segs([(0, 96, 128), (1, 0, 64)]),
    2: _segs([(1, 64, 128), (2, 0, 32)]),
    3: _segs([(2, 32, 128)]),
}

LAST_RESULTS = None
_CACHED_NC = None


def _u_perm():
    p = []
    for h in range(HEADS):
        p += list(range(h * HD, (h + 1) * HD))
        p += list(range(DIM + h * HD, DIM + (h + 1) * HD))
    return np.array(p)


def _bf16(a):
    return np.ascontiguousarray(a.astype(ml_dtypes.bfloat16))


def build_nc():
    nc = bacc.Bacc("TRN2", target_bir_lowering=False, debug=False,
                   enable_asserts=False, num_devices=N_CORES)
    xb = nc.dram_tensor("xb", [B, DIM, NF], BF16, kind="ExternalInput").ap()
    yb = nc.dram_tensor("yb", [B, DIM, NF], BF16, kind="ExternalInput").ap()
    wqk = nc.dram_tensor("wqk", [DIM, 2 * DIM], BF16, kind="ExternalInput").ap()
    aqk = nc.dram_tensor("aqk", [3, 128, 9], F32, kind="ExternalInput").ap()
    ddw = nc.dram_tensor("ddw", [128, 9, 128], BF16, kind="ExternalInput").ap()
    wconv = nc.dram_tensor("wconv", [9, DIM, DIM], BF16, kind="ExternalInput").ap()
    projth = nc.dram_tensor("projth", [HEADS, HD, DIM], BF16, kind="ExternalInput").ap()
    tempt = nc.dram_tensor("tempt", [HD, HEADS], F32, kind="ExternalInput").ap()
    eye = nc.dram_tensor("eye", [UD, UD], F32, kind="ExternalInput").ap()
    out = nc.dram_tensor("out", [B, DIM, NI], BF16, kind="ExternalOutput").ap()

    with tile.TileContext(nc) as tc:
        with tc.tile_pool(name="wpool", bufs=1) as wp, \
             tc.tile_pool(name="xs", bufs=2) as xsp, \
             tc.tile_pool(name="mid", bufs=1) as midp, \
             tc.tile_pool(name="yt", bufs=1) as ytp, \
             tc.tile_pool(name="qk", bufs=1) as qkp, \
             tc.tile_pool(name="dwt", bufs=1) as dwp, \
             tc.tile_pool(name="ut", bufs=2) as utp, \
             tc.tile_pool(name="vv", bufs=1) as vvp, \
             tc.tile_pool(name="small", bufs=1) as smp, \
             tc.tile_pool(name="ost", bufs=3) as ostp, \
             tc.tile_pool(name="psA", bufs=4, space="PSUM") as psA, \
             tc.tile_pool(name="psB", bufs=2, space="PSUM") as psB, \
             tc.tile_pool(name="psG", bufs=1, space="PSUM") as psG, \
             tc.tile_pool(name="psS", bufs=1, space="PSUM") as psS, \
             tc.tile_pool(name="dram", bufs=2, space="DRAM") as drp:

            # ---- persistent weights ----
            wqk_a = wp.tile([128, 2 * DIM], BF16, tag="wqk_a")
            wqk_b = wp.tile([64, 2 * DIM], BF16, tag="wqk_b")
            nc.sync.dma_start(wqk_a[:], wqk[0:128, :])
            nc.sync.dma_start(wqk_b[:], wqk[128:192, :])
            wca = wp.tile([128, 9, DIM], BF16, tag="wca")
            wcb = wp.tile([64, 9, DIM], BF16, tag="wcb")
            nc.sync.dma_start(wca[:], wconv[:, 0:128, :].rearrange("t k m -> k t m"))
            nc.sync.dma_start(wcb[:],
                              wconv[:, 128:192, :].rearrange("t k m -> k t m"))
            aqk_sb = wp.tile([128, 3, 9], F32, tag="aqk")
            nc.sync.dma_start(aqk_sb[:], aqk.rearrange("c k t -> k c t"))
            ddw_sb = wp.tile([128, 9, 128], BF16, tag="ddw")
            nc.sync.dma_start(ddw_sb[:], ddw[:])
            projth_sb = wp.tile([HD, HEADS, DIM], BF16, tag="projth")
            nc.sync.dma_start(projth_sb[:], projth.rearrange("h d f -> d h f"))
            tempt_sb = wp.tile([HD, HEADS], F32, tag="tempt")
            nc.sync.dma_start(tempt_sb[:], tempt[:])
            eye_sb = wp.tile([UD, UD], F32, tag="eye")
            nc.sync.dma_start(eye_sb[:], eye[:])
            ones_sb = wp.tile([UD, UD], F32, tag="ones")
            nc.vector.memset(ones_sb[:], 1.0)

            GRP = 512
            groups = [(g, min(g + GRP, NF)) for g in range(0, NF, GRP)]

            # guarded tiles are allocated once and reused for both batches;
            # guard columns stay zero (interior writes never touch them).
            mids = []
            for ct in range(3):
                mid = midp.tile([128, RIN, WG], BF16, tag=f"mid{ct}")
                nc.gpsimd.memset(mid[:, :, 0:1], 0.0)
                nc.gpsimd.memset(mid[:, :, WG - 1:WG], 0.0)
                mids.append(mid)
            ya = ytp.tile([128, RIN, WG], BF16, tag="ya")
            ybt = ytp.tile([64, RIN, WG], BF16, tag="ybt")
            nc.gpsimd.memset(ya[:, :, 0:1], 0.0)
            nc.gpsimd.memset(ya[:, :, WG - 1:WG], 0.0)
            nc.gpsimd.memset(ybt[:, :, 0:1], 0.0)
            nc.gpsimd.memset(ybt[:, :, WG - 1:WG], 0.0)

            def qk_mm_phase(b):
                """1x1 conv into 3 guarded mid tiles (PE + scalar evac)."""
                for (g0, g1) in groups:
                    x_a = xsp.tile([128, GRP], BF16, tag="x_a")
                    x_b = xsp.tile([64, GRP], BF16, tag="x_b")
                    nc.sync.dma_start(x_a[:, 0:g1 - g0], xb[b, 0:128, g0:g1])
                    nc.sync.dma_start(x_b[:, 0:g1 - g0], xb[b, 128:192, g0:g1])
                    for nb in range(g0 // NB, g1 // NB):
                        ns = slice(nb * NB - g0, (nb + 1) * NB - g0)
                        pss = [psA.tile([128, NB], F32, tag="psA",
                                        name=f"psqk{m}") for m in range(3)]
                        for m in range(3):
                            mcol = slice(m * 128, (m + 1) * 128)
                            nc.tensor.matmul(pss[m][:], wqk_a[:, mcol],
                                             x_a[:, ns], start=True, stop=False)
                        for m in range(3):
                            mcol = slice(m * 128, (m + 1) * 128)
                            nc.tensor.matmul(pss[m][:], wqk_b[:, mcol],
                                             x_b[:, ns], start=False, stop=True)
                        r = nb * 2
                        for m in range(3):
                            nc.scalar.copy(
                                mids[m][:, r:r + 2, 1:W + 1],
                                pss[m][:].rearrange("p (r w) -> p r w", r=2))

            def dw_pe_phase(b):
                """dx=0 depthwise taps as diagonal matmuls; scalar evac is
                the qt init."""
                qkt = []
                for ct in range(3):
                    qt = qkp.tile([128, ROWS, W], BF16, tag=f"qk{ct}")
                    for j in range(NCH):
                        psD = psB.tile([128, NB], F32, tag="psB", name="psD")
                        for k, t in enumerate(PE_TAPS):
                            dy = TAPS[t][0]
                            rs = slice(1 + 2 * j + dy, 3 + 2 * j + dy)
                            nc.tensor.matmul(psD[:], ddw_sb[:, ct * 3 + k, :],
                                             mids[ct][:, rs, 1:W + 1],
                                             start=(k == 0), stop=(k == 2))
                        nc.scalar.copy(
                            qt[:, 2 * j:2 * j + 2, :],
                            psD[:].rearrange("p (r w) -> p r w", r=2))
                    qkt.append(qt)
                return qkt

            def dw_dve_phase(b, qkt):
                """6 aligned depthwise taps on DVE: ts_mul + tt_add.
                hf-outer so mid rows free progressively (next batch's 1x1
                evacs unblock early) and qt bands complete in order."""
                for hf in range(4):
                    ro = slice(hf * 8, hf * 8 + 8)
                    for ct in range(3):
                        mid, qt = mids[ct], qkt[ct]
                        dwt = dwp.tile([128, ROWS // 4, W], BF16, tag="dwt")
                        for t in DVE_TAPS:
                            dy, dx = TAPS[t]
                            rs = slice(1 + hf * 8 + dy, 9 + hf * 8 + dy)
                            src = mid[:, rs, 1 + dx:1 + W + dx]
                            nc.vector.tensor_scalar_mul(
                                dwt[:], src, aqk_sb[:, ct, t:t + 1])
                            nc.vector.tensor_tensor(
                                qt[:, ro, :], dwt[:], qt[:, ro, :], op=ADD)

            def conv_load(b, eng):
                """y band DMAs; eng picks the issue queue so batch 1 does
                not serialize behind AR-gated DMAs on the sync queue."""
                eng.dma_start(
                    ya[:, :, 1:W + 1],
                    yb[b, 0:128, :].rearrange("c (r w) -> c r w", r=RIN))
                eng.dma_start(
                    ybt[:, :, 1:W + 1],
                    yb[b, 128:192, :].rearrange("c (r w) -> c r w", r=RIN))

            def win(j, t):
                dy, dx = TAPS[t]
                return (slice(1 + 2 * j + dy, 3 + 2 * j + dy),
                        slice(1 + dx, 1 + W + dx))

            def conv_a_phase(b):
                """3x3 full conv y -> v, out channels 0:128 per chunk."""
                v0 = vvp.tile([128, NI], BF16, tag="v0")
                for j in range(NCH):
                    ns = slice(j * NB, (j + 1) * NB)
                    psP = psA.tile([128, NB], F32, tag="psA", name="psP")
                    for t in range(9):
                        rs, cs = win(j, t)
                        nc.tensor.matmul(psP[:], wca[:, t, 0:128],
                                         ya[:, rs, cs],
                                         start=(t == 0), stop=False)
                    for t in range(9):
                        rs, cs = win(j, t)
                        nc.tensor.matmul(psP[:], wcb[:, t, 0:128],
                                         ybt[:, rs, cs],
                                         start=False, stop=(t == 8))
                    nc.scalar.copy(v0[:, ns], psP[:])
                return v0

            def conv_b_phase(b):
                """out channels 128:192, chunk pairs col-packed."""
                vB = vvp.tile([128, NI // 2], BF16, tag="vB")
                for p in range(NCH // 2):
                    j0, j1 = 2 * p, 2 * p + 1
                    pns = slice(p * NB, (p + 1) * NB)
                    psR = psB.tile([128, NB], F32, tag="psB", name="psR")
                    for t in range(9):
                        rs0, cs0 = win(j0, t)
                        rs1, cs1 = win(j1, t)
                        nc.tensor.matmul(psR[0:64, :], wca[:, t, 128:192],
                                         ya[:, rs0, cs0],
                                         start=(t == 0), stop=False)
                        nc.tensor.matmul(psR[64:128, :], wca[:, t, 128:192],
                                         ya[:, rs1, cs1],
                                         start=(t == 0), stop=False)
                    for t in range(9):
                        rs0, cs0 = win(j0, t)
                        rs1, cs1 = win(j1, t)
                        nc.tensor.matmul(psR[0:64, :], wcb[:, t, 128:192],
                                         ybt[:, rs0, cs0],
                                         start=False, stop=(t == 8))
                        nc.tensor.matmul(psR[64:128, :], wcb[:, t, 128:192],
                                         ybt[:, rs1, cs1],
                                         start=False, stop=(t == 8))
                    nc.scalar.copy(vB[:, pns], psR[:])
                return vB

            def gram_ar_phase(b, qkt):
                """Per-head u-gram + norms; DVE-free (scalar/gpsimd/PE)."""
                gu_all = smp.tile([UD, HEADS, UD], F32, tag="gu_all")
                ncol = smp.tile([UD, HEADS], F32, tag="ncol")
                n_bands = NI // 2048
                tp_i = 0
                for h in range(HEADS):
                    gps = psG.tile([128, UD], F32, tag="psG")
                    for band in range(n_bands):
                        rsl = slice(band * 8, (band + 1) * 8)
                        ut = utp.tile([128, 16, 128], BF16, tag="ut")
                        off = 0
                        for (ct, lo, hi) in HEAD_SEGS[h]:
                            eng = nc.sync if tp_i % 2 == 0 else nc.scalar
                            eng.dma_start_transpose(
                                ut[:, :, off:off + hi - lo],
                                qkt[ct][lo:hi, rsl, :])
                            tp_i += 1
                            off += hi - lo
                        for c in range(16):
                            nc.tensor.matmul(
                                gps[:], ut[:, c, :], ut[:, c, 0:UD],
                                start=(band == 0 and c == 0),
                                stop=(band == n_bands - 1 and c == 15))
                    nc.scalar.copy(gu_all[:, h, :], gps[0:UD, :])
                # qk block + diag norms. tmp = gu .* I is diagonal, so
                # tmp @ ones-col gives the norms as a per-partition column.
                aru = smp.tile([HD, HEADS, HD], F32, tag="aru")
                tmp = smp.tile([UD, UD], F32, tag="tmp")
                for h in range(HEADS):
                    nc.scalar.copy(aru[:, h, :], gu_all[0:HD, h, HD:UD])
                    nc.gpsimd.tensor_tensor(
                        tmp[:], gu_all[:, h, :], eye_sb[:], op=MULT)
                    nps = psS.tile([UD, 1], F32, tag="psS", name="nps")
                    nc.tensor.matmul(nps[:], tmp[:], ones_sb[:, 0:1],
                                     start=True, stop=True)
                    nc.scalar.copy(ncol[:, h:h + 1], nps[:])
                # AR payload packed to [48, 200]: qk blocks in cols 0:192,
                # norms in cols 192:200 as two stacked [48, 4] blocks.
                QW = HEADS * HD
                ar_in = drp.tile([HD, QW + 8], F32, tag="ar_in")
                ar_out = drp.tile([HD, QW + 8], F32, tag="ar_out")
                nc.sync.dma_start(ar_in[:, 0:QW],
                                  aru[:].rearrange("p h d -> p (h d)"))
                nc.sync.dma_start(ar_in[:, QW:QW + 4], ncol[0:HD, :])
                nc.sync.dma_start(ar_in[:, QW + 4:QW + 8], ncol[HD:UD, :])
                nc.gpsimd.collective_compute(
                    "AllReduce", ADD,
                    replica_groups=[list(range(N_CORES))],
                    ins=[ar_in.opt()], outs=[ar_out.opt()])
                gqk = smp.tile([HD, HEADS, HD], F32, tag="gqk")
                nsqt = smp.tile([HD, 2, HEADS], F32, tag="nsqt")
                nc.sync.dma_start(gqk[:].rearrange("p h d -> p (h d)"),
                                  ar_out[:, 0:QW])
                nc.sync.dma_start(nsqt[:, 0, :], ar_out[:, QW:QW + 4])
                nc.sync.dma_start(nsqt[:, 1, :], ar_out[:, QW + 4:QW + 8])
                return gqk, nsqt

            def post_phase(b, gqk, nsqt):
                """Softmax + proj fold; DVE use limited to 1 tiny
                reciprocal pair (queued after the dw blocks)."""
                # invu[c, u, h] = 1/||u||: reciprocal on DVE (scalar Rsqrt
                # is banned for accuracy), sqrt on Scalar.
                inq = smp.tile([HD, 2, HEADS], F32, tag="inq")
                nc.vector.reciprocal(inq[:], nsqt[:])
                invu = smp.tile([HD, 2, HEADS], F32, tag="invu")
                nc.scalar.sqrt(invu[:], inq[:])
                invq = smp.tile([HD, HEADS], F32, tag="invq")
                nc.gpsimd.tensor_tensor(invq[:], invu[:, 0, :], tempt_sb[:],
                                        op=MULT)
                exs = smp.tile([HD, HEADS, HD], BF16, tag="exs")
                sm_all = smp.tile([HD, HEADS], F32, tag="sm_all")
                for h in range(HEADS):
                    # k-norm column -> row (PE transpose), then broadcast
                    # down 48 partitions with a K=1 ones matmul.
                    ktp = psS.tile([1, HD], F32, tag="psS", name="ktp")
                    nc.tensor.transpose(ktp[:], invu[:, 1, h:h + 1],
                                        eye_sb[0:HD, 0:HD])
                    krow = smp.tile([1, HD], F32, tag="krow")
                    nc.scalar.copy(krow[:], ktp[:])
                    bc = smp.tile([HD, HD], F32, tag="bc")
                    bps = psS.tile([HD, HD], F32, tag="psS", name="bps")
                    nc.tensor.matmul(bps[:], ones_sb[0:1, 0:HD], krow[:],
                                     start=True, stop=True)
                    nc.scalar.copy(bc[:], bps[:])
                    lg = smp.tile([HD, HD], F32, tag="lg")
                    nc.gpsimd.tensor_scalar_mul(
                        lg[:], gqk[:, h, :], invq[:, h:h + 1])
                    nc.gpsimd.tensor_tensor(lg[:], lg[:], bc[:], op=MULT)
                    # logits bounded by temperature (cosine sims): no max
                    # subtraction needed; fused row-sum via accum_out.
                    nc.scalar.activation(exs[:, h, :], lg[:],
                                         mybir.ActivationFunctionType.Exp,
                                         accum_out=sm_all[:, h:h + 1])
                rs_all = smp.tile([HD, HEADS], F32, tag="rs_all")
                nc.vector.reciprocal(rs_all[:], sm_all[:])
                mst = smp.tile([HD, HEADS, DIM], BF16, tag="mst")
                for h in range(HEADS):
                    mps = psS.tile([HD, DIM], F32, tag="psS", name="mps")
                    nc.tensor.matmul(mps[:], exs[:, h, :], projth_sb[:, h, :],
                                     start=True, stop=True)
                    # softmax 1/rowsum folded into the PSUM evacuation
                    nc.scalar.mul(mst[:, h, :], mps[:], rs_all[:, h:h + 1])
                pe0 = smp.tile([128, DIM], BF16, tag="pe0")
                pe1 = smp.tile([128, DIM], BF16, tag="pe1")
                nc.sync.dma_start(pe0[0:48, :], mst[:, 0, :])
                nc.sync.dma_start(pe0[48:96, :], mst[:, 1, :])
                nc.sync.dma_start(pe0[96:128, :], mst[0:32, 2, :])
                # pe1 duplicated on both partition halves so the K=64
                # final matmuls can run at base partition 0 or 64 to
                # match the col-packed vB layout.
                for base in (0, 64):
                    nc.sync.dma_start(pe1[base:base + 16, :], mst[32:48, 2, :])
                    nc.sync.dma_start(pe1[base + 16:base + 64, :], mst[:, 3, :])
                return pe0, pe1

            def final_phase(b, pe0, pe1, v0, vB):
                """out = PeffT.T @ v.  psF0: out 0:128 per chunk.  psF2:
                out 128:192 col-packed chunk pairs."""
                for p in range(NCH // 2):
                    j0, j1 = 2 * p, 2 * p + 1
                    pns = slice(p * NB, (p + 1) * NB)
                    ns0 = slice(j0 * NB, (j0 + 1) * NB)
                    ns1 = slice(j1 * NB, (j1 + 1) * NB)
                    psF0a = psA.tile([128, NB], F32, tag="psA", name="psF0a")
                    psF0b = psA.tile([128, NB], F32, tag="psA", name="psF0b")
                    psF2 = psB.tile([128, NB], F32, tag="psB", name="psF2")
                    nc.tensor.matmul(psF0a[:], pe0[:, 0:128], v0[:, ns0],
                                     start=True, stop=False)
                    nc.tensor.matmul(psF0a[:], pe1[0:64, 0:128], vB[0:64, pns],
                                     start=False, stop=True)
                    nc.tensor.matmul(psF0b[:], pe0[:, 0:128], v0[:, ns1],
                                     start=True, stop=False)
                    nc.tensor.matmul(psF0b[:], pe1[64:128, 0:128],
                                     vB[64:128, pns], start=False, stop=True)
                    nc.tensor.matmul(psF2[0:64, :], pe0[:, 128:192], v0[:, ns0],
                                     start=True, stop=False)
                    nc.tensor.matmul(psF2[0:64, :], pe1[0:64, 128:192],
                                     vB[0:64, pns], start=False, stop=True)
                    nc.tensor.matmul(psF2[64:128, :], pe0[:, 128:192],
                                     v0[:, ns1], start=True, stop=False)
                    nc.tensor.matmul(psF2[64:128, :], pe1[64:128, 128:192],
                                     vB[64:128, pns], start=False, stop=True)
                    ost0 = ostp.tile([128, NB], BF16, tag="ost")
                    ost1 = ostp.tile([128, NB], BF16, tag="ost")
                    ost2 = ostp.tile([128, NB], BF16, tag="ost")
                    nc.scalar.copy(ost0[:], psF0a[:])
                    nc.scalar.copy(ost1[:], psF0b[:])
                    nc.scalar.copy(ost2[:], psF2[:])
                    nc.sync.dma_start(out[b, 0:128, ns0], ost0[:])
                    nc.sync.dma_start(out[b, 0:128, ns1], ost1[:])
                    nc.sync.dma_start(out[b, 128:192, ns0], ost2[0:64, :])
                    nc.sync.dma_start(out[b, 128:192, ns1], ost2[64:128, :])

            # ---- global schedule ----
            # Per-engine queues are in-order; the DVE queue holds the two
            # dw blocks back-to-back (plus post's 3 tiny reciprocals).
            # qk1/dw1 are sandwiched between conv0's passes so dw1 and
            # hence gram1/AR1 start as early as possible; conv pass B of
            # batch 0 fills the PE during AR0's latency; final0 precedes
            # conv1 so v (bufs=1) can be reused.
            qk_mm_phase(0)
            qkt0 = dw_pe_phase(0)
            dw_dve_phase(0, qkt0)
            conv_load(0, nc.sync)
            v00 = conv_a_phase(0)
            vB0 = conv_b_phase(0)
            qk_mm_phase(1)
            gqk0, nt0 = gram_ar_phase(0, qkt0)
            qkt1 = dw_pe_phase(1)
            dw_dve_phase(1, qkt1)
            conv_load(1, nc.gpsimd)
            pe00, pe10 = post_phase(0, gqk0, nt0)
            final_phase(0, pe00, pe10, v00, vB0)
            v01 = conv_a_phase(1)
            vB1 = conv_b_phase(1)
            gqk1, nt1 = gram_ar_phase(1, qkt1)
            pe01, pe11 = post_phase(1, gqk1, nt1)
            final_phase(1, pe01, pe11, v01, vB1)

    nc.finalize()
    return nc


def _host_prep(inputs):
    x = np.asarray(inputs["x"], dtype=np.float32)
    y = np.asarray(inputs["y"], dtype=np.float32)
    qk_w = np.asarray(inputs["qk_w"], dtype=np.float32)[:, :, 0, 0]
    qk_dw = np.asarray(inputs["qk_dw_w"], dtype=np.float32)[:, 0]
    v_w = np.asarray(inputs["v_w"], dtype=np.float32)[:, :, 0, 0]
    v_dw = np.asarray(inputs["v_dw_w"], dtype=np.float32)
    proj = np.asarray(inputs["proj_w"], dtype=np.float32)[:, :, 0, 0]
    temp = np.asarray(inputs["temperature"], dtype=np.float32).reshape(HEADS)

    perm = _u_perm()
    wqk_l = _bf16(qk_w[perm].T)                              # [192, 384]
    aqk_t = np.ascontiguousarray(
        qk_dw[perm].reshape(3, 128, 9).astype(np.float32))
    # diagonal weight matrices for the PE depthwise taps (dx == 0)
    ddw_t = np.zeros((128, 9, 128), np.float32)
    for ct in range(3):
        for k, t in enumerate(PE_TAPS):
            np.fill_diagonal(ddw_t[:, ct * 3 + k, :], aqk_t[ct, :, t])
    # fold v 1x1 into the 3x3: v = Conv3x3(y; W_dw@W_v), lhsT layout [in,out]
    wconv_l = _bf16(np.stack(
        [(v_dw[:, :, dy + 1, dx + 1] @ v_w).T for dy, dx in TAPS]))
    projth = _bf16(np.stack(
        [proj[:, h * HD:(h + 1) * HD].T for h in range(HEADS)]))  # [4,48,192]
    tempt = np.ascontiguousarray(
        np.broadcast_to(temp[None, :], (HD, HEADS)).astype(np.float32))
    eye = np.eye(UD, dtype=np.float32)

    # halo-padded row bands per core, bf16
    xp = np.pad(x, ((0, 0), (0, 0), (1, 1), (0, 0)))
    yp = np.pad(y, ((0, 0), (0, 0), (1, 1), (0, 0)))
    shared = dict(wqk=wqk_l, aqk=aqk_t, ddw=_bf16(ddw_t), wconv=wconv_l,
                  projth=projth, tempt=tempt, eye=eye)
    in_maps = []
    for c in range(N_CORES):
        rs = slice(c * ROWS, c * ROWS + RIN)
        in_maps.append(dict(
            xb=_bf16(xp[:, :, rs]).reshape(B, DIM, NF),
            yb=_bf16(yp[:, :, rs]).reshape(B, DIM, NF),
            **shared))
    return in_maps


def kernel(**inputs):
    global LAST_RESULTS, _CACHED_NC
    in_maps = _host_prep(inputs)
    if _CACHED_NC is None:
        _CACHED_NC = build_nc()
    res = run_bass_kernel_spmd(
        _CACHED_NC, in_maps, core_ids=list(range(N_CORES)))
    LAST_RESULTS = res
    out = np.empty((B, DIM, H, W), np.float32)
    for c in range(N_CORES):
        band = res.results[c]["out"].astype(np.float32).reshape(B, DIM, ROWS, W)
        out[:, :, c * ROWS:(c + 1) * ROWS] = band
    return out
